# revision 51
# baseline (speedup 1.0000x reference)
"""AdderNet 2D convolution on 8 TRN2 NeuronCores.

out[n,co,h,w] = -sum_{ci,kh,kw} |xpad[n,ci,h+kh,w+kw] - w[co,ci,kh,kw]|

Sharding: data-parallel over the batch dim (16 images -> 2 per core),
weight replicated.  No collectives needed (forward pass only).

Math: |x - w| = x + w - 2*min(x, w), so

  -sum |x - w| = 2*sum min(x, w) - sum x - sum w

The heavy term is one single-op DVE tensor_scalar per (co, tap):
y = min(xpad, w[co,ci,kh,kw]) in bf16 (4x perf mode), evaluated over the
full padded plane so every instruction is contiguous/aligned.  TensorE
reduces partitions with a constant +2 block stationary into PSUM
(accumulating the 9 taps); the (kh,kw) tap shift is applied by the
matmul's strided moving-view.  "sum x" is accumulated by 252 extra
matmuls with an all-(-1) stationary; "sum w" comes in as a tiny
host-precomputed per-partition bias, applied in the epilogue.

Per-core layout:
  - 128 SBUF partitions = img*64 + ci  (2 images per core)
  - psum/output partition p = 32*(co//16) + 2*(co%16) + img
    (TensorE column-tiling: 4 strips of 32, one per co-group)
  - zero padding in xpad contributes min(0, w) terms and the matching
    zeros in sum x, exactly reproducing the reference's |0 - w| border
    terms.
"""

import numpy as np

try:
    from concourse import bacc, mybir, tile
except ImportError:  # pragma: no cover - fallback when sitecustomize absent
    import sys

    sys.path.insert(0, "/opt/trn_rl_repo")
    from concourse import bacc, mybir, tile

from concourse.bass_utils import run_bass_kernel_spmd

N, C, H, W = 16, 64, 56, 56
CO, K = 64, 3
NCORES = 8
NLOC = N // NCORES  # images per core = 2
HP = H + 2  # padded plane height
WP = W + 2
L = H * W  # 3136 output pixels
CHUNK_ROWS = 8  # output rows per psum bank chunk
NCHUNK = H // CHUNK_ROWS  # 7
CHUNK = CHUNK_ROWS * W  # 448 <= 512 fp32 / psum bank

_nc_cache = None


def build_nc():
    nc = bacc.Bacc(
        "TRN2",
        target_bir_lowering=False,
        debug=False,
        num_devices=NCORES,
    )
    f32 = mybir.dt.float32
    bf16 = mybir.dt.bfloat16

    # host-padded bf16 plane: x[p = img*64 + ci, (h, w) of the 58x58
    # zero-bordered image]
    x_d = nc.dram_tensor("x", [128, HP * WP], bf16, kind="ExternalInput")
    xs_d = nc.dram_tensor("xs", [128, HP * WP], bf16, kind="ExternalInput")
    # host-preshuffled weights: wb[p=img*64+ci, co*9+t] = w[co, ci, t]
    wb_d = nc.dram_tensor("wb", [128, CO * K * K], f32, kind="ExternalInput")
    wn_d = nc.dram_tensor("wn", [128, CO * K * K], f32, kind="ExternalInput")
    # swn[p, 0] = -sum_{ci, min-taps} w[co(p)] at psum partition p (host side)
    swn_d = nc.dram_tensor("swn", [128, 1], f32, kind="ExternalInput")
    # out rows are psum-partition-major: p = 32*(co//16) + 2*(co%16) + img;
    # the host-side gather untangles this ordering (cheap numpy transpose).
    o_d = nc.dram_tensor("out", [128, L], f32, kind="ExternalOutput")

    with tile.TileContext(nc) as tc:
        with (
            tc.tile_pool(name="const", bufs=1) as cpool,
            tc.tile_pool(name="ypool", bufs=12) as ypool,
            tc.tile_pool(name="apool", bufs=16) as apool,
            tc.tile_pool(name="psum", bufs=1, space="PSUM") as ppool,
        ):
            xpad = cpool.tile([128, HP, WP], bf16)
            # xpadB[p, r, c] = xpad[p, r, c+1]: left-shifted copy so the
            # kw==1 DVE taps read from a 4-byte-aligned window (keeps the
            # tensor_scalar in 4x perf mode).
            xpadB = cpool.tile([128, HP, WP], bf16)
            wbias = cpool.tile([128, CO * K * K], f32)
            wneg = cpool.tile([128, CO * K * K], f32)
            swn = cpool.tile([128, 1], f32)
            # stat2[:, c, :]: [128, 32] stationary, col 2c+i = +2 on the
            # img-i partition half, else 0  (the 2*min reduction).
            stat2 = cpool.tile([128, 16, 32], bf16)
            # statm[:, c, :]: same pattern with -1 (the -|x-w| ACT tiles).
            statm = cpool.tile([128, 16, 32], bf16)
            # statn: [128, 32] all-columns -1 on matching img half (sum-x).
            statn = cpool.tile([128, 32], bf16)
            # statx6/statx7: like statn but only over the co columns for
            # which that tap runs on the DVE (min-trick) path.
            statx6 = cpool.tile([128, 32], bf16)
            statx7 = cpool.tile([128, 32], bf16)
            out_sb = cpool.tile([128, L], f32)

            # ---- loads -------------------------------------------------
            # contiguous DMA of the host-padded planes, split across rings
            xpflat = xpad[:].rearrange("p h w -> p (h w)")
            xsflat = xpadB[:].rearrange("p h w -> p (h w)")
            dma_engs = [nc.sync, nc.gpsimd, nc.scalar, nc.sync,
                        nc.gpsimd, nc.scalar, nc.sync, nc.gpsimd]
            for q in range(4):
                dma_engs[q].dma_start(
                    xpflat[32 * q : 32 * (q + 1), :],
                    x_d.ap()[32 * q : 32 * (q + 1), :],
                )
                dma_engs[q + 4].dma_start(
                    xsflat[32 * q : 32 * (q + 1), :],
                    xs_d.ap()[32 * q : 32 * (q + 1), :],
                )
            nc.gpsimd.dma_start(wbias[:], wb_d.ap())
            nc.gpsimd.dma_start(wneg[:], wn_d.ap())
            nc.sync.dma_start(swn[:], swn_d.ap())

            # ---- constants --------------------------------------------
            nc.vector.memset(stat2[:], 0.0)
            nc.vector.memset(statm[:], 0.0)
            for c in range(16):
                nc.vector.memset(stat2[0:64, c, 2 * c : 2 * c + 1], 2.0)
                nc.vector.memset(stat2[64:128, c, 2 * c + 1 : 2 * c + 2], 2.0)
                nc.vector.memset(statm[0:64, c, 2 * c : 2 * c + 1], -1.0)
                nc.vector.memset(statm[64:128, c, 2 * c + 1 : 2 * c + 2], -1.0)
            nc.vector.memset(statn[:], 0.0)
            nc.vector.memset(statn[0:64, 0:32:2], -1.0)
            nc.vector.memset(statn[64:128, 1:32:2], -1.0)
            nc.vector.memset(statx6[:], 0.0)
            for c in range(16):
                if c not in (2, 4, 7, 9, 11, 14):
                    nc.vector.memset(statx6[0:64, 2 * c : 2 * c + 1], -1.0)
                    nc.vector.memset(statx6[64:128, 2 * c + 1 : 2 * c + 2], -1.0)
            nc.vector.memset(statx7[:], 0.0)
            nc.vector.memset(statx7[0:64, 0:1], -1.0)
            nc.vector.memset(statx7[64:128, 1:2], -1.0)

            wb2 = wbias[:]
            wn2 = wneg[:]

            psums = [
                ppool.tile([128, CHUNK], f32, name=f"ps{f}", tag=f"ps{f}")
                for f in range(NCHUNK)
            ]

            taps = [(kh, kw) for kh in range(K) for kw in range(K)]
            # act-tap distribution (total 36 tap-slots = 144 act tiles):
            # cc0 gets a single act tap (8,) so the first PE block is not
            # gated on the ScalarE warm-up; ACT3 blocks get three.
            ACT3 = (2, 4, 7, 9, 11, 14)
            def act_taps(cc):
                if cc == 0:
                    return (8,)
                return (6, 7, 8) if cc in ACT3 else (7, 8)

            def tap_src(t):
                """[128, 56, 56] window of the padded plane for tap t,
                4B-aligned for the DVE (odd kw reads the shifted copy)."""
                kh, kw = taps[t]
                if kw == 1:
                    return xpadB[:, kh : kh + H, 0:W]
                return xpad[:, kh : kh + H, kw : kw + W]


            # start-flag bookkeeping: first MM to touch each (g, chunk)
            # PSUM region must carry start=True (emission order == PE order)
            started = [[False] * NCHUNK for _ in range(4)]

            def mm(f, g, lhsT, rhs, stop=False):
                st = not started[g][f]
                started[g][f] = True
                nc.tensor.matmul(
                    psums[f][32 * g : 32 * g + 32, :],
                    lhsT,
                    rhs,
                    start=st,
                    stop=stop,
                    tile_position=(0, 32 * g),
                )

            # sum-x matmul groups, spread through the cc blocks below
            sumx_groups = [(t, f) for t in range(8) for f in range(NCHUNK)]

            def emit_sumx(t, f):
                kh, kw = taps[t]
                r0 = f * CHUNK_ROWS
                rhs = xpad[:, r0 + kh : r0 + kh + CHUNK_ROWS, kw : kw + W]
                lhsT = {6: statx6[:], 7: statx7[:]}.get(t, statn[:])
                for g in range(4):
                    mm(f, g, lhsT, rhs)

            # ---- main loop -------------------------------------------
            # DVE tiles: y = min(x, w)   (contiguous output) -> stationary +2
            # ACT tiles: y = |x - w|                         -> stationary -1
            # ACT taps emitted first within each cc block: ScalarE runs one
            # cc ahead (separate pool), so its tiles are ready when the PE
            # stream reaches the act matmuls.
            for cc in range(16):
                lo = (len(sumx_groups) * cc) // 16
                hi = (len(sumx_groups) * (cc + 1)) // 16
                for t, f in sumx_groups[lo:hi]:
                    emit_sumx(t, f)
                at = act_taps(cc)
                mins = [t for t in range(9) if t not in at]
                if cc == 0:
                    # first block: ACT ramps up behind the DMA; consume its
                    # tile at the END of the block.
                    tap_order = mins + list(at)
                else:
                    # interleave act-consuming groups between min groups so
                    # the PE never needs a long run of banked ACT tiles
                    tap_order = []
                    step = len(mins) // len(at) + 1
                    ai, mi = 0, 0
                    for k in range(9):
                        if k % step == 1 and ai < len(at):
                            tap_order.append(at[ai]); ai += 1
                        elif mi < len(mins):
                            tap_order.append(mins[mi]); mi += 1
                        elif ai < len(at):
                            tap_order.append(at[ai]); ai += 1
                for t in tap_order:
                    is_act = t in at
                    for g in range(4):
                        co = 16 * g + cc
                        if not is_act:
                            y = ypool.tile([128, L], bf16, tag="y")
                            nc.vector.tensor_scalar(
                                y[:].rearrange("p (a b) -> p a b", b=W),
                                tap_src(t),
                                wb2[:, co * 9 + t : co * 9 + t + 1],
                                None,
                                op0=mybir.AluOpType.min,
                            )
                            lhsT = stat2[:, cc, :]
                        else:
                            y = apool.tile([128, L], bf16, tag="ya")
                            nc.scalar.activation(
                                y[:].rearrange("p (a b) -> p a b", b=W),
                                tap_src(t),
                                mybir.ActivationFunctionType.Abs,
                                bias=wn2[:, co * 9 + t : co * 9 + t + 1],
                            )
                            lhsT = statm[:, cc, :]
                        for f in range(NCHUNK):
                            rhs = y[:, f * CHUNK : (f + 1) * CHUNK]
                            mm(
                                f, g, lhsT, rhs,
                                stop=(cc == 15 and t == tap_order[-1]),
                            )

            # ---- epilogue: out = psum + (-sum w), psum -> sbuf -> dram --
            # per-chunk copy + DMA so the tail overlaps the psum drain
            out_engs = [nc.sync, nc.gpsimd, nc.scalar]
            for f in range(NCHUNK):
                nc.vector.tensor_scalar(
                    out_sb[:, f * CHUNK : (f + 1) * CHUNK],
                    psums[f][:],
                    swn[:],
                    None,
                    op0=mybir.AluOpType.add,
                )
                out_engs[f % 3].dma_start(
                    o_d.ap()[:, f * CHUNK : (f + 1) * CHUNK],
                    out_sb[:, f * CHUNK : (f + 1) * CHUNK],
                )

    nc.compile()
    return nc


def get_nc():
    global _nc_cache
    if _nc_cache is None:
        _nc_cache = build_nc()
    return _nc_cache


def make_in_maps(x, w):
    import ml_dtypes

    xb = np.ascontiguousarray(x).astype(ml_dtypes.bfloat16)
    xpad_all = np.zeros((N * C, HP, WP), dtype=ml_dtypes.bfloat16)
    xpad_all[:, 1 : H + 1, 1 : W + 1] = xb.reshape(N * C, H, W)
    xpad_all = xpad_all.reshape(N, C, HP * WP)
    xs_all = np.zeros_like(xpad_all)
    xs_all[:, :, 0 : HP * WP - 1] = xpad_all[:, :, 1:]
    w = np.ascontiguousarray(w, dtype=np.float32)
    wr = w.reshape(CO, C, K * K)
    # wb[p = img*64 + ci, co*9 + t] = w[co, ci, t]  (both img halves)
    wb_half = wr.transpose(1, 0, 2).reshape(C, CO * K * K)
    wb = np.ascontiguousarray(np.vstack([wb_half, wb_half]), dtype=np.float32)
    wn = np.ascontiguousarray(-wb)
    # -sum w[co] over each co's min-trick taps: t 0..5 always; t6 when
    # cc=co%16 not in ACT3={2,5,8,11,14}; t7 only when cc==0; t8 never.
    # Scattered to psum partitions p = 32*(co//16)+2*(co%16)+img.
    swc = -wr[:, :, :6].sum(axis=(1, 2))
    mask6 = np.array([(co % 16) not in (2, 4, 7, 9, 11, 14) for co in range(CO)])
    swc -= np.where(mask6, wr[:, :, 6].sum(axis=1), 0.0)
    mask7 = np.array([(co % 16) == 0 for co in range(CO)])
    swc -= np.where(mask7, wr[:, :, 7].sum(axis=1), 0.0)
    swn = np.empty((128, 1), dtype=np.float32)
    for co in range(CO):
        p = 32 * (co // 16) + 2 * (co % 16)
        swn[p, 0] = swc[co]
        swn[p + 1, 0] = swc[co]
    return [
        {
            "x": np.ascontiguousarray(
                xpad_all[i * NLOC : (i + 1) * NLOC].reshape(128, HP * WP)
            ),
            "xs": np.ascontiguousarray(
                xs_all[i * NLOC : (i + 1) * NLOC].reshape(128, HP * WP)
            ),
            "wb": wb,
            "wn": wn,
            "swn": swn,
        }
        for i in range(NCORES)
    ]


def unscramble(core_out):
    """[128, L] with row p = 32*(co//16) + 2*(co%16) + img -> [2, 64, 56, 56]."""
    return (
        core_out.reshape(4, 16, NLOC, H, W)
        .transpose(2, 0, 1, 3, 4)
        .reshape(NLOC, CO, H, W)
    )


def kernel(x, w):
    nc = get_nc()
    res = run_bass_kernel_spmd(nc, make_in_maps(x, w), core_ids=list(range(NCORES)))
    out = np.concatenate([unscramble(r["out"]) for r in res.results], axis=0)
    return np.ascontiguousarray(out, dtype=np.float32)


if __name__ == "__main__":
    x = np.random.randn(N, C, H, W).astype(np.float32)
    w = np.random.randn(CO, C, K, K).astype(np.float32)
    o = kernel(x, w)
    print("out", o.shape, o.dtype, float(o.mean()))


# revision 52
# speedup vs baseline: 1.1790x; 1.1790x over previous
"""AdderNet 2D convolution on 8 TRN2 NeuronCores.

out[n,co,h,w] = -sum_{ci,kh,kw} |xpad[n,ci,h+kh,w+kw] - w[co,ci,kh,kw]|

Sharding: data-parallel over the batch dim (16 images -> 2 per core),
weight replicated.  No collectives needed (forward pass only).

Math: |x - w| = x + w - 2*min(x, w), so

  -sum |x - w| = 2*sum min(x, w) - sum x - sum w

The heavy term is one single-op DVE tensor_scalar per (co, tap):
y = min(xpad, w[co,ci,kh,kw]) in bf16 (4x perf mode), evaluated over the
full padded plane so every instruction is contiguous/aligned.  TensorE
reduces partitions with a constant +2 block stationary into PSUM
(accumulating the 9 taps); the (kh,kw) tap shift is applied by the
matmul's strided moving-view.  "sum x" is accumulated by 252 extra
matmuls with an all-(-1) stationary; "sum w" comes in as a tiny
host-precomputed per-partition bias, applied in the epilogue.

Per-core layout:
  - 128 SBUF partitions = img*64 + ci  (2 images per core)
  - psum/output partition p = 32*(co//16) + 2*(co%16) + img
    (TensorE column-tiling: 4 strips of 32, one per co-group)
  - zero padding in xpad contributes min(0, w) terms and the matching
    zeros in sum x, exactly reproducing the reference's |0 - w| border
    terms.
"""

import numpy as np

try:
    from concourse import bacc, mybir, tile
except ImportError:  # pragma: no cover - fallback when sitecustomize absent
    import sys

    sys.path.insert(0, "/opt/trn_rl_repo")
    from concourse import bacc, mybir, tile

from concourse.bass_utils import run_bass_kernel_spmd

N, C, H, W = 16, 64, 56, 56
CO, K = 64, 3
NCORES = 8
NLOC = N // NCORES  # images per core = 2
HP = H + 2  # padded plane height
WP = W + 2
L = H * W  # 3136 output pixels
CHUNK_ROWS = 8  # output rows per psum bank chunk
NCHUNK = H // CHUNK_ROWS  # 7
CHUNK = CHUNK_ROWS * W  # 448 <= 512 fp32 / psum bank

_nc_cache = None


def build_nc():
    nc = bacc.Bacc(
        "TRN2",
        target_bir_lowering=False,
        debug=False,
        num_devices=NCORES,
    )
    f32 = mybir.dt.float32
    bf16 = mybir.dt.bfloat16

    # host-padded bf16 plane: x[p = img*64 + ci, (h, w) of the 58x58
    # zero-bordered image]
    x_d = nc.dram_tensor("x", [128, HP * WP], bf16, kind="ExternalInput")
    xs_d = nc.dram_tensor("xs", [128, HP * WP], bf16, kind="ExternalInput")
    # host-preshuffled weights: wb[p=img*64+ci, co*9+t] = w[co, ci, t]
    wb_d = nc.dram_tensor("wb", [128, CO * K * K], f32, kind="ExternalInput")
    wn_d = nc.dram_tensor("wn", [128, CO * K * K], f32, kind="ExternalInput")
    # swn[p, 0] = -sum_{ci, min-taps} w[co(p)] at psum partition p (host side)
    swn_d = nc.dram_tensor("swn", [128, 1], f32, kind="ExternalInput")
    # out rows are psum-partition-major: p = 32*(co//16) + 2*(co%16) + img;
    # the host-side gather untangles this ordering (cheap numpy transpose).
    o_d = nc.dram_tensor("out", [128, L], f32, kind="ExternalOutput")

    with tile.TileContext(nc) as tc:
        with (
            tc.tile_pool(name="const", bufs=1) as cpool,
            tc.tile_pool(name="ypool", bufs=12) as ypool,
            tc.tile_pool(name="apool", bufs=16) as apool,
            tc.tile_pool(name="psum", bufs=1, space="PSUM") as ppool,
        ):
            xpad = cpool.tile([128, HP, WP], bf16)
            # xpadB[p, r, c] = xpad[p, r, c+1]: left-shifted copy so the
            # kw==1 DVE taps read from a 4-byte-aligned window (keeps the
            # tensor_scalar in 4x perf mode).
            xpadB = cpool.tile([128, HP, WP], bf16)
            wbias = cpool.tile([128, CO * K * K], f32)
            wneg = cpool.tile([128, CO * K * K], f32)
            swn = cpool.tile([128, 1], f32)
            # stat2[:, c, :]: [128, 32] stationary, col 2c+i = +2 on the
            # img-i partition half, else 0  (the 2*min reduction).
            stat2 = cpool.tile([128, 16, 32], bf16)
            # statm[:, c, :]: same pattern with -1 (the -|x-w| ACT tiles).
            statm = cpool.tile([128, 16, 32], bf16)
            # statn: [128, 32] all-columns -1 on matching img half (sum-x).
            statn = cpool.tile([128, 32], bf16)
            # statx6/statx7: like statn but only over the co columns for
            # which that tap runs on the DVE (min-trick) path.
            statx6 = cpool.tile([128, 32], bf16)
            statx7 = cpool.tile([128, 32], bf16)
            out_sb = cpool.tile([128, L], f32)

            # ---- loads -------------------------------------------------
            # contiguous DMA of the host-padded planes, split across rings
            xpflat = xpad[:].rearrange("p h w -> p (h w)")
            xsflat = xpadB[:].rearrange("p h w -> p (h w)")
            dma_engs = [nc.sync, nc.gpsimd, nc.scalar, nc.sync,
                        nc.gpsimd, nc.scalar, nc.sync, nc.gpsimd]
            for q in range(4):
                dma_engs[q].dma_start(
                    xpflat[32 * q : 32 * (q + 1), :],
                    x_d.ap()[32 * q : 32 * (q + 1), :],
                )
                dma_engs[q + 4].dma_start(
                    xsflat[32 * q : 32 * (q + 1), :],
                    xs_d.ap()[32 * q : 32 * (q + 1), :],
                )
            nc.gpsimd.dma_start(wbias[:], wb_d.ap())
            nc.gpsimd.dma_start(wneg[:], wn_d.ap())
            nc.sync.dma_start(swn[:], swn_d.ap())

            # ---- constants --------------------------------------------
            nc.vector.memset(stat2[:], 0.0)
            nc.vector.memset(statm[:], 0.0)
            for c in range(16):
                nc.vector.memset(stat2[0:64, c, 2 * c : 2 * c + 1], 2.0)
                nc.vector.memset(stat2[64:128, c, 2 * c + 1 : 2 * c + 2], 2.0)
                nc.vector.memset(statm[0:64, c, 2 * c : 2 * c + 1], -1.0)
                nc.vector.memset(statm[64:128, c, 2 * c + 1 : 2 * c + 2], -1.0)
            nc.vector.memset(statn[:], 0.0)
            nc.vector.memset(statn[0:64, 0:32:2], -1.0)
            nc.vector.memset(statn[64:128, 1:32:2], -1.0)
            nc.vector.memset(statx6[:], 0.0)
            for c in range(16):
                if c not in (2, 4, 7, 9, 11, 14):
                    nc.vector.memset(statx6[0:64, 2 * c : 2 * c + 1], -1.0)
                    nc.vector.memset(statx6[64:128, 2 * c + 1 : 2 * c + 2], -1.0)
            nc.vector.memset(statx7[:], 0.0)
            nc.vector.memset(statx7[0:64, 0:1], -1.0)
            nc.vector.memset(statx7[64:128, 1:2], -1.0)

            wb2 = wbias[:]
            wn2 = wneg[:]

            psums = [
                ppool.tile([128, CHUNK], f32, name=f"ps{f}", tag=f"ps{f}")
                for f in range(NCHUNK)
            ]

            taps = [(kh, kw) for kh in range(K) for kw in range(K)]
            # act-tap distribution (total 36 tap-slots = 144 act tiles):
            # cc0 gets a single act tap (8,) so the first PE block is not
            # gated on the ScalarE warm-up; ACT3 blocks get three.
            ACT3 = (2, 4, 7, 9, 11, 14)
            def act_taps(cc):
                if cc == 0:
                    return (8,)
                return (6, 7, 8) if cc in ACT3 else (7, 8)

            def tap_src(t):
                """[128, 56, 56] window of the padded plane for tap t,
                4B-aligned for the DVE (odd kw reads the shifted copy)."""
                kh, kw = taps[t]
                if kw == 1:
                    return xpadB[:, kh : kh + H, 0:W]
                return xpad[:, kh : kh + H, kw : kw + W]


            # start-flag bookkeeping: first MM to touch each (g, chunk)
            # PSUM region must carry start=True (emission order == PE order)
            started = [[False] * NCHUNK for _ in range(4)]

            def mm(f, g, lhsT, rhs, stop=False):
                st = not started[g][f]
                started[g][f] = True
                nc.tensor.matmul(
                    psums[f][32 * g : 32 * g + 32, :],
                    lhsT,
                    rhs,
                    start=st,
                    stop=stop,
                    tile_position=(0, 32 * g),
                )

            # sum-x matmul groups, spread through the cc blocks below
            sumx_groups = [(t, f) for t in range(8) for f in range(NCHUNK)]

            def emit_sumx(t, f):
                kh, kw = taps[t]
                r0 = f * CHUNK_ROWS
                rhs = xpad[:, r0 + kh : r0 + kh + CHUNK_ROWS, kw : kw + W]
                lhsT = {6: statx6[:], 7: statx7[:]}.get(t, statn[:])
                for g in range(4):
                    mm(f, g, lhsT, rhs)

            # ---- main loop -------------------------------------------
            # DVE tiles: y = min(x, w)   (contiguous output) -> stationary +2
            # ACT tiles: y = |x - w|                         -> stationary -1
            # ACT taps emitted first within each cc block: ScalarE runs one
            # cc ahead (separate pool), so its tiles are ready when the PE
            # stream reaches the act matmuls.
            for cc in range(16):
                lo = (len(sumx_groups) * cc) // 16
                hi = (len(sumx_groups) * (cc + 1)) // 16
                for t, f in sumx_groups[lo:hi]:
                    emit_sumx(t, f)
                at = act_taps(cc)
                mins = [t for t in range(9) if t not in at]
                if cc == 0:
                    # first block: ACT ramps up behind the DMA; consume its
                    # tile at the END of the block.
                    tap_order = mins + list(at)
                else:
                    # interleave act-consuming groups between min groups so
                    # the PE never needs a long run of banked ACT tiles
                    tap_order = []
                    step = len(mins) // len(at) + 1
                    ai, mi = 0, 0
                    for k in range(9):
                        if k % step == 1 and ai < len(at):
                            tap_order.append(at[ai]); ai += 1
                        elif mi < len(mins):
                            tap_order.append(mins[mi]); mi += 1
                        elif ai < len(at):
                            tap_order.append(at[ai]); ai += 1
                for t in tap_order:
                    is_act = t in at
                    for g in range(4):
                        co = 16 * g + cc
                        if not is_act:
                            y = ypool.tile([128, L], bf16, tag="y")
                            nc.vector.tensor_scalar(
                                y[:].rearrange("p (a b) -> p a b", b=W),
                                tap_src(t),
                                wb2[:, co * 9 + t : co * 9 + t + 1],
                                None,
                                op0=mybir.AluOpType.min,
                            )
                            lhsT = stat2[:, cc, :]
                        else:
                            y = apool.tile([128, L], bf16, tag="ya")
                            nc.scalar.activation(
                                y[:].rearrange("p (a b) -> p a b", b=W),
                                tap_src(t),
                                mybir.ActivationFunctionType.Abs,
                                bias=wn2[:, co * 9 + t : co * 9 + t + 1],
                            )
                            lhsT = statm[:, cc, :]
                        for f in range(NCHUNK):
                            rhs = y[:, f * CHUNK : (f + 1) * CHUNK]
                            mm(
                                f, g, lhsT, rhs,
                                stop=(cc == 15 and t == tap_order[-1]),
                            )

            # ---- epilogue: out = psum + (-sum w), psum -> sbuf -> dram --
            for f in range(NCHUNK):
                nc.vector.tensor_scalar(
                    out_sb[:, f * CHUNK : (f + 1) * CHUNK],
                    psums[f][:],
                    swn[:],
                    None,
                    op0=mybir.AluOpType.add,
                )
            out_engs = [nc.sync, nc.gpsimd, nc.scalar, nc.sync]
            for q in range(4):
                out_engs[q].dma_start(
                    o_d.ap()[32 * q : 32 * (q + 1), :],
                    out_sb[32 * q : 32 * (q + 1), :],
                )

    nc.compile()
    return nc


def get_nc():
    global _nc_cache
    if _nc_cache is None:
        _nc_cache = build_nc()
    return _nc_cache


def make_in_maps(x, w):
    import ml_dtypes

    xb = np.ascontiguousarray(x).astype(ml_dtypes.bfloat16)
    xpad_all = np.zeros((N * C, HP, WP), dtype=ml_dtypes.bfloat16)
    xpad_all[:, 1 : H + 1, 1 : W + 1] = xb.reshape(N * C, H, W)
    xpad_all = xpad_all.reshape(N, C, HP * WP)
    xs_all = np.zeros_like(xpad_all)
    xs_all[:, :, 0 : HP * WP - 1] = xpad_all[:, :, 1:]
    w = np.ascontiguousarray(w, dtype=np.float32)
    wr = w.reshape(CO, C, K * K)
    # wb[p = img*64 + ci, co*9 + t] = w[co, ci, t]  (both img halves)
    wb_half = wr.transpose(1, 0, 2).reshape(C, CO * K * K)
    wb = np.ascontiguousarray(np.vstack([wb_half, wb_half]), dtype=np.float32)
    wn = np.ascontiguousarray(-wb)
    # -sum w[co] over each co's min-trick taps: t 0..5 always; t6 when
    # cc=co%16 not in ACT3={2,5,8,11,14}; t7 only when cc==0; t8 never.
    # Scattered to psum partitions p = 32*(co//16)+2*(co%16)+img.
    swc = -wr[:, :, :6].sum(axis=(1, 2))
    mask6 = np.array([(co % 16) not in (2, 4, 7, 9, 11, 14) for co in range(CO)])
    swc -= np.where(mask6, wr[:, :, 6].sum(axis=1), 0.0)
    mask7 = np.array([(co % 16) == 0 for co in range(CO)])
    swc -= np.where(mask7, wr[:, :, 7].sum(axis=1), 0.0)
    swn = np.empty((128, 1), dtype=np.float32)
    for co in range(CO):
        p = 32 * (co // 16) + 2 * (co % 16)
        swn[p, 0] = swc[co]
        swn[p + 1, 0] = swc[co]
    return [
        {
            "x": np.ascontiguousarray(
                xpad_all[i * NLOC : (i + 1) * NLOC].reshape(128, HP * WP)
            ),
            "xs": np.ascontiguousarray(
                xs_all[i * NLOC : (i + 1) * NLOC].reshape(128, HP * WP)
            ),
            "wb": wb,
            "wn": wn,
            "swn": swn,
        }
        for i in range(NCORES)
    ]


def unscramble(core_out):
    """[128, L] with row p = 32*(co//16) + 2*(co%16) + img -> [2, 64, 56, 56]."""
    return (
        core_out.reshape(4, 16, NLOC, H, W)
        .transpose(2, 0, 1, 3, 4)
        .reshape(NLOC, CO, H, W)
    )


def kernel(x, w):
    nc = get_nc()
    res = run_bass_kernel_spmd(nc, make_in_maps(x, w), core_ids=list(range(NCORES)))
    out = np.concatenate([unscramble(r["out"]) for r in res.results], axis=0)
    return np.ascontiguousarray(out, dtype=np.float32)


if __name__ == "__main__":
    x = np.random.randn(N, C, H, W).astype(np.float32)
    w = np.random.randn(CO, C, K, K).astype(np.float32)
    o = kernel(x, w)
    print("out", o.shape, o.dtype, float(o.mean()))
